# revision 23
# baseline (speedup 1.0000x reference)
"""GCN block (2-layer) Trainium2 Bass kernel, v3.

Math (per B*T slice, shared graph):
  t2 = relu(A @ (X @ W1) + b1);  out = sigmoid(A @ t2 @ W2 + b2)
  A = D^-1/2 (Adj + I) D^-1/2  (PyG gcn_norm, counts edge multiplicity)

Device mapping (all-fp8 PE pipeline, M = Adj + I exact small ints in fp8):
  W1 : DoubleRow K=256 over slice-QUADS - stationary = X quad blocks
       [128=(h,cin), r, node], moving = blockdiag4(W1) [128, r, 256].
       One matmul produces a full 256-wide (2-pl) output chunk with the
       contraction finished, so W1 PE work halves vs the K=128 form.
       PSUM rotates banks 0-4; drains split DVE/ACT/Pool 3 ways (the
       drain engines, not PE, pace this era together with the X DMA).
  ck0: checkpoint-0's L1 runs DURING W1 as a 3-chain subpass (banks
       5-7, block h2=0) consuming xwp pairs a few steps behind
       production; block h2=1 runs as a second 3-chain subpass right
       after.  This fills the X-DMA-paced W1 era with L1 work.
  L1 : ckpts 1-4 - 6 chains on banks 0-5 in two trios (skew 0/1) so
       drains land 3-wide on DVE/Pool/ACT and xwp slots release at
       round i+1 for the t2c read handoff.  MT streams in HALF-chunk
       tiles (ring of 3) with natural pair order (half A pairs 0-19
       first), which shrinks the MT ring 40->30 KB/partition.
  t2 : one AllGather per checkpoint; 8 early t2c reads (group 0) land
       in the retired xb ring + a dedicated escape pool during ckpts
       1-3; groups 1-3 read into xwp slots as ckpt4 releases them;
       group 4 (half-A first) right after the last exchange.  The
       first L2 pass consumes pairs in ORD (arrival) order, so reads
       stay ahead of the chains and the read stall ~vanishes.
  L2 : FLIPPED orientation - stationary = t2 pair tiles (node-major),
       moving = MT dst-chunk slabs, fp8 DoubleRow; 12 chains run as 6
       co-banked PAIRS (pl 2p/2p+1 share bank p, same skew p, same end
       round, so the full bank drains at once - no half-bank-drain
       hazard).  W2 + sigmoid run on banks 6/7 inside the pass tail.
  W2 : stationary blockdiag(W2) bf16 over the drained bf16 s2 chunks,
       sigmoid+bias on ACT, fp32 tiles DMA'd to the output.

Sharding: each of 8 cores owns 10 of the 80 dst-node blocks (N padded
10000->10240) for ALL 24 B*T slices.  SBUF keeps the full xw / t2
operand set resident (40 pair tiles, 120 KB/partition); the same ring
is reused between layers (t2c tiles overwrite xwp slots).
"""
import time

import numpy as np
import ml_dtypes

import concourse.bacc as bacc
import concourse.mybir as mybir
import concourse.tile as tile
from concourse.bass_utils import run_bass_kernel_spmd

N_CORES = 8
N = 10000
NP = 10240            # padded nodes
NB = NP // 128        # 80 node blocks
NB2 = NB // 2         # 40 src-block pairs (DoubleRow K=256)
HNB2 = NB2 // 2       # 20 pairs per MT half tile
BPC = NB // N_CORES   # 10 dst blocks per core
NCK = BPC // 2        # 5 t2 checkpoints (dst-block pairs) per core
CHW = 256             # L2 dst-chunk width
NCH = BPC * 128 // CHW  # 5 dst chunks per core
B, T, C = 2, 12, 64
S = B * T             # 24 slices
F = S * C             # 1536 free columns
PAIRS = S // 2        # 12 slice pairs (pl)
QUADS = S // 4        # 6 slice quads (W1 DoubleRow groups)
CHAINS = ((0, 512), (512, 512), (1024, 512))
W1SCALE = 8.0         # W1 pre-scale so fp8 weights stay mostly normal
CK0LAG = 4            # ckpt0 subpass-A lag behind W1 production

f32 = mybir.dt.float32
bf16 = mybir.dt.bfloat16
fp8 = mybir.dt.float8e4
DR = mybir.MatmulPerfMode.DoubleRow
AF = mybir.ActivationFunctionType

# W1 drain engine rotation: ACT ~498ns, DVE ~595ns, Pool ~711ns per
# [128,512] chunk -> weights 7:6:5 balance the three engines.
ENG18 = "ADPADPADPADPADADAP"


def build_program(with_collective=True, nc_hook=None):
    nc = bacc.Bacc("TRN2", target_bir_lowering=False, debug=False,
                   num_devices=N_CORES)
    if nc_hook is not None:
        nc_hook(nc)

    # X quad blocks: [b][128=(h,cin)][sq*256 + r*128 + node] fp8,
    # dinv-src folded; slice s = 4*sq + 2*r + h
    xb_ext = nc.dram_tensor("XB8", [NB, 128, F], fp8, kind="ExternalInput")
    # MT column slabs: [chunk][128 src][j2*512 + k*256 + dst] fp8 ints
    mt_ext = nc.dram_tensor("MT", [NCH, 128, NB2 * 2 * CHW], fp8,
                            kind="ExternalInput")
    # blockdiag4(W1*8) packed for DoubleRow: [128=(h,cin), r*256 + col]
    w1_ext = nc.dram_tensor("W1d", [128, 512], fp8, kind="ExternalInput")
    w2_ext = nc.dram_tensor("W2d", [128, 128], bf16, kind="ExternalInput")
    b1_ext = nc.dram_tensor("B1", [128, 512], f32, kind="ExternalInput")
    b2_ext = nc.dram_tensor("B2", [128, 1], f32, kind="ExternalInput")
    di_ext = nc.dram_tensor("DI", [128, BPC], f32, kind="ExternalInput")
    d8_ext = nc.dram_tensor("DI8", [128, BPC], f32, kind="ExternalInput")
    db_ext = nc.dram_tensor("DB", [128, BPC * 128], f32,
                            kind="ExternalInput")
    out_ext = nc.dram_tensor("OUT", [PAIRS, 128, BPC * 128], f32,
                             kind="ExternalOutput")

    with tile.TileContext(nc) as tc:
        with (
            tc.tile_pool(name="consts", bufs=1) as consts,
            tc.tile_pool(name="xb", bufs=4) as pool_xb,
            tc.tile_pool(name="xwp", bufs=NB2) as pool_xwp,
            tc.tile_pool(name="esc", bufs=8) as pool_esc,
            tc.tile_pool(name="mt", bufs=2) as pool_mt,
            tc.tile_pool(name="u", bufs=2) as pool_u,
            tc.tile_pool(name="t2s", bufs=2) as pool_t2s,
            tc.tile_pool(name="s2", bufs=8) as pool_s2,
            tc.tile_pool(name="outst", bufs=3) as pool_out,
            tc.tile_pool(name="ps", bufs=8, space="PSUM") as pool_ps,
            tc.tile_pool(name="dram", bufs=1, space="DRAM") as dram,
        ):
            # startup-critical const only; the small drain consts ride
            # the ACT queue so the SP HWDGE goes straight to X blocks
            w1t = consts.tile([128, 2, 256], fp8, tag="w1")
            nc.sync.dma_start(w1t[:].rearrange("p r c -> p (r c)"),
                              w1_ext[:])
            b1t = consts.tile([128, 512], f32, tag="b1")
            nc.scalar.dma_start(b1t[:], b1_ext[:])
            dit = consts.tile([128, BPC], f32, tag="di")
            nc.scalar.dma_start(dit[:], di_ext[:])
            di8 = consts.tile([128, BPC], f32, tag="di8")
            nc.scalar.dma_start(di8[:], d8_ext[:])

            # DRAM intermediates: per-checkpoint t2 slabs
            t2loc = [dram.tile([2, 128, F], fp8, tag="t2loc",
                               name=f"t2loc{k}") for k in range(NCK)]
            if with_collective:
                t2full = [dram.tile([N_CORES, 2, 128, F], fp8, tag="t2full",
                                    name=f"t2full{k}", addr_space="Shared")
                          for k in range(NCK)]
            else:
                t2full = [dram.tile([N_CORES, 2, 128, F], fp8, tag="t2full",
                                    name=f"t2full{k}") for k in range(NCK)]

            # t2c handoff order: pairs grouped by checkpoint (j2 % NCK)
            ORD = [k + NCK * i for k in range(NCK) for i in range(NB2 // NCK)]

            def ps_tile(slot, name):
                return pool_ps.tile([128, 512], f32, tag=f"s{slot}",
                                    bufs=1, name=name)

            def mt_load(ch, half, eng=None):
                t = pool_mt.tile([128, HNB2, 2, CHW], fp8, tag="mt",
                                 name=f"mt{ch}{'AB'[half]}")
                off = half * HNB2 * 2 * CHW
                (eng or nc.sync).dma_start(
                    t[:].rearrange("p a b d -> p (a b d)"),
                    mt_ext[ch][:, off:off + HNB2 * 2 * CHW])
                return t

            def mt_pair(mth, i):
                # [128, 2, 256] slab for pair i from half tiles
                return mth[i // HNB2][:, i % HNB2, :, :]

            # xwp tiles ALLOCATED in ORD order so slot releases (which
            # fire in allocation order) line up with ckpt4's ORD-order
            # consumption and the t2c reads that reuse the slots.
            xwp = [None] * NB2
            for i in range(NB2):
                xwp[ORD[i]] = pool_xwp.tile([128, 2, F], fp8, tag="xwp",
                                            name=f"xwp{ORD[i]}")

            exchanged = []

            def exchange(ch, t2st):
                nc.gpsimd.dma_start(
                    t2loc[ch][:].rearrange("a p f -> p a f"), t2st[:])
                if with_collective:
                    nc.gpsimd.collective_compute(
                        "AllGather", mybir.AluOpType.bypass,
                        replica_groups=[list(range(N_CORES))],
                        ins=[t2loc[ch][:]], outs=[t2full[ch][:]])
                else:
                    # one 8-way broadcast DMA (0-stride src) models the
                    # same per-core traffic as the 8 slab writes but
                    # costs a single descriptor-gen pass
                    nc.gpsimd.dma_start(
                        t2full[ch][:],
                        t2loc[ch][:].unsqueeze(0)
                        .broadcast_to([N_CORES, 2, 128, F]))
                exchanged.append(ch)

            t2c = [None] * NB2

            def t2c_read(j2, t):
                gc = 2 * j2
                core, ck = gc // BPC, (gc % BPC) // 2
                assert ck in exchanged, (j2, ck)
                t2c[j2] = t
                nc.sync.dma_start(
                    t[:], t2full[ck][core].rearrange("a p f -> p a f"))

            # ---- W1 (DoubleRow), X-DMA paced -----------------------
            # Per block, 6 quad matmuls [128,256] with the contraction
            # done in one DR pass; PSUM rotates all 8 banks as
            # [128,512] (2-quad) chunks; drains split DVE/ACT/Pool.
            # No other PE work is interleaved: the era is X-DMA bound
            # and extra work here would run at the mid p-state (2x).
            nchunk = 0
            # ckpt0 half A loads ahead of the X stream (+3.6us startup,
            # but the stream then runs uninterrupted); with the 2-deep
            # MT ring every later load is WAR-timed by its slot, so the
            # scheduler cannot hoist it into a critical DMA window.
            mth = {0: [mt_load(0, 0), None]}
            for j2 in range(NB2):
                xb = pool_xb.tile([128, 2, F], fp8, tag="xb",
                                  name=f"xb{j2}")
                nc.sync.dma_start(
                    xb[:], xb_ext[2 * j2:2 * j2 + 2]
                    .rearrange("a p d -> p a d"))
                xw = xwp[j2]
                for k in range(2):
                    for sq2 in range(3):
                        ps = ps_tile(nchunk % 8, f"w1p{nchunk}")
                        for q in range(2):
                            sq = 2 * sq2 + q
                            nc.tensor.matmul(
                                ps[:, q * 256:(q + 1) * 256],
                                xb[:, k, sq * 256:(sq + 1) * 256]
                                .rearrange("p (r n) -> p r n", r=2),
                                w1t[:], start=True, stop=True,
                                perf_mode=DR)
                        dst = xw[:, k, sq2 * 512:(sq2 + 1) * 512]
                        e = ENG18[nchunk % 18]
                        if e == "A":
                            nc.scalar.activation(dst, ps[:], AF.Copy)
                        elif e == "D":
                            nc.vector.tensor_scalar_mul(dst, ps[:], 1.0)
                        else:
                            nc.gpsimd.tensor_scalar_mul(dst, ps[:], 1.0)
                        nchunk += 1

            mth[0][1] = mt_load(0, 1)

            # ---- L1 ckpts 0-4: 6 chains as two trios (skew 0/1) -----
            for ch in range(NCK):
                t2st = pool_t2s.tile([128, 2, F], fp8, tag="t2s",
                                     name=f"t2s{ch}")
                order = (list(range(NB2)) if ch < NCK - 1
                         else [ORD[i] for i in range(NB2)])
                ps_list = [ps_tile(idx, f"pa{ch}_{idx}")
                           for idx in range(6)]
                for r in range(NB2 + 2):
                    for idx in range(6):
                        h2, skew = idx // 3, idx // 3
                        i = r - skew
                        if not 0 <= i < NB2:
                            continue
                        j2 = order[i]
                        c0, w = CHAINS[idx % 3]
                        nc.tensor.matmul(
                            ps_list[idx][:],
                            mt_pair(mth[ch], j2)[:, :,
                                                 h2 * 128:(h2 + 1) * 128],
                            xwp[j2][:, :, c0:c0 + w],
                            start=(i == 0), stop=(i == NB2 - 1),
                            perf_mode=DR)
                    if ch == NCK - 1 and 12 <= r - 2 < 32:
                        # slot ORD[r-2] released by trio B last round;
                        # read groups 1-3 into the freed xwp slots.
                        i2 = r - 2
                        j2r = ORD[i2]
                        t = pool_xwp.tile([128, 2, F], fp8, tag="xwp",
                                          name=f"t2c{j2r}")
                        t2c_read(j2r, t)
                    if ch < NCK - 1 and r == 21:
                        # next ckpt's half A: its ring slot's readers
                        # (this ckpt's half-A rounds) are emitted, so
                        # the WAR edge binds and times the load.
                        mth[ch + 1] = [mt_load(ch + 1, 0), None]
                    for idx in range(6):
                        h2, skew = idx // 3, idx // 3
                        if r - skew != NB2 - 1:
                            continue
                        c0, w = CHAINS[idx % 3]
                        bi = 2 * ch + h2
                        u = pool_u.tile([128, 512], f32, tag="u")
                        eng = nc.vector if idx % 3 != 1 else nc.gpsimd
                        eng.scalar_tensor_tensor(
                            u[:], ps_list[idx][:], di8[:, bi:bi + 1],
                            b1t[:], mybir.AluOpType.mult,
                            mybir.AluOpType.add)
                        nc.scalar.activation(t2st[:, h2, c0:c0 + w], u[:],
                                             AF.Relu,
                                             scale=dit[:, bi:bi + 1])
                if ch < NCK - 1:
                    mth[ch + 1][1] = mt_load(ch + 1, 1)
                exchange(ch, t2st)
                if ch <= 1:
                    # 10 early t2c reads: group 0 (8) after exchange 0
                    # into the retired xb ring + escape pool; the first
                    # two group-1 pairs after exchange 1.  They fire as
                    # soon as their exchange lands, over ckpts 1-3.
                    for m in (range(8) if ch == 0 else range(8, 12)):
                        j2e = ORD[m]
                        if m < 4:
                            t = pool_xb.tile([128, 2, F], fp8, tag="xb",
                                             name=f"t2ce{j2e}")
                        else:
                            t = pool_esc.tile([128, 2, F], fp8, tag="esc",
                                              name=f"t2ce{j2e}")
                        t2c_read(j2e, t)
                if ch == 0:
                    # L2-era consts ride the ckpt1-window DMA slack
                    w2t = consts.tile([128, 128], bf16, tag="w2")
                    nc.sync.dma_start(w2t[:], w2_ext[:])
                    b2t = consts.tile([128, 1], f32, tag="b2")
                    nc.sync.dma_start(b2t[:], b2_ext[:])
                    dbt = consts.tile([128, BPC * 128], f32, tag="db")
                    nc.sync.dma_start(dbt[:], db_ext[:])

            # group-4 t2c reads (half-A pairs first = ORD order), into
            # the xwp slots ckpt4 released (allocation order continues)
            for i2 in range(32, NB2):
                j2r = ORD[i2]
                t = pool_xwp.tile([128, 2, F], fp8, tag="xwp",
                                  name=f"t2c{j2r}")
                t2c_read(j2r, t)

            # ---- L2 (flipped) + W2 + sigmoid, per dst chunk ---------
            # 12 chains = 6 co-banked pairs (pl 2p/2p+1 on bank p, skew
            # p, same end round -> full-bank drain).  W2 on banks 6/7.
            # Chunk NCH-1 first (MT resident, pairs in ORD/arrival
            # order); then descending with natural halfwise order and
            # half-granular MT reloads.
            for ch in range(NCH - 1, -1, -1):
                if ch == NCH - 1:
                    mtt = mth[NCK - 1]     # still resident from L1
                    order = [ORD[i] for i in range(NB2)]
                else:
                    order = list(range(NB2))
                    mtt = mth[ch]
                wt = [ps_tile(p, f"l2w{ch}_{p}") for p in range(6)]
                s2l = [None] * PAIRS
                for r in range(NB2 + 6):
                    for p in range(6):
                        i = r - p
                        if not 0 <= i < NB2:
                            continue
                        j2 = order[i]
                        for h in range(2):
                            pl = 2 * p + h
                            nc.tensor.matmul(
                                wt[p][:, h * CHW:(h + 1) * CHW],
                                t2c[j2][:, :, pl * 128:(pl + 1) * 128],
                                mt_pair(mtt, j2)[:],
                                start=(i == 0), stop=(i == NB2 - 1),
                                perf_mode=DR)
                    p = r - (NB2 - 1)
                    if 0 <= p < 6:
                        # full-bank ready: drain both halves (s2 = bf16
                        # dinv_dst fold); W2 is deferred past the pass
                        for h in range(2):
                            pl = 2 * p + h
                            s2 = pool_s2.tile([128, CHW], bf16, tag="s2")
                            eng = nc.vector if h == 0 else nc.gpsimd
                            eng.tensor_tensor(
                                s2[:], wt[p][:, h * CHW:(h + 1) * CHW],
                                dbt[:, ch * CHW:(ch + 1) * CHW],
                                mybir.AluOpType.mult)
                            s2l[pl] = s2
                # deferred W2 + sigmoid block: the W2 psums reuse the
                # chain banks (drained above), so no W2 matmul ever
                # stalls the in-order PE stream mid-pass, and the next
                # chunk's chains WAR only on the sigmoids.
                for p in range(6):
                    w2ps = ps_tile(p, f"w2p{ch}_{p}")
                    for h in range(2):
                        nc.tensor.matmul(w2ps[:, h * CHW:(h + 1) * CHW],
                                         w2t[:], s2l[2 * p + h][:],
                                         start=True, stop=True)
                    # one full-bank sigmoid per pl-pair (b2 bias is
                    # per-partition, identical for both halves)
                    outst = pool_out.tile([128, 2, CHW], f32, tag="outst")
                    nc.scalar.activation(
                        outst[:].rearrange("p a d -> p (a d)"), w2ps[:],
                        AF.Sigmoid, bias=b2t[:])
                    nc.sync.dma_start(
                        out_ext[2 * p:2 * p + 2, :,
                                ch * CHW:(ch + 1) * CHW]
                        .rearrange("a p d -> p a d"), outst[:])
                if ch > 0:
                    # reload next chunk's halves; emitted after the full
                    # round loop so every reader of the recycled ring
                    # slots exists before the WAR edge is formed.  The
                    # ACT queue pins them behind this chunk's sigmoids,
                    # clear of the t2c read stream.
                    mth[ch - 1] = [mt_load(ch - 1, 0), mt_load(ch - 1, 1)]

    nc.compile()
    return nc


def prepare_inputs(X, edge_index, W1, b1, W2, b2):
    """Host-side graph/layout prep. Returns per-core in_maps."""
    X = np.asarray(X, dtype=np.float32)
    edge_index = np.asarray(edge_index)
    W1 = np.asarray(W1, dtype=np.float32)
    b1 = np.asarray(b1, dtype=np.float32)
    W2 = np.asarray(W2, dtype=np.float32)
    b2 = np.asarray(b2, dtype=np.float32)

    src = edge_index[0].astype(np.int64)
    dst = edge_index[1].astype(np.int64)

    deg = np.bincount(dst, minlength=N).astype(np.float32) + 1.0
    dinv = 1.0 / np.sqrt(deg)
    dinv_pad = np.zeros(NP, np.float32)
    dinv_pad[:N] = dinv

    # M = Adj + I with multiplicity, uint8 counts
    Mfull = np.zeros((NP, NP), np.uint8)
    np.add.at(Mfull, (dst, src), 1)
    Mfull[np.arange(N), np.arange(N)] += 1
    assert Mfull.max() <= 15, "fp8e4 exact-int range exceeded"

    # XB: [NB, 128=(h,cin), sq*256 + r*128 + node] fp8, dinv-src folded;
    # slice s = 4*sq + 2*r + h
    Xs = X * dinv[None, :, None, None]                  # [B, N, T, C]
    XT = np.zeros((S, C, NP), np.float32)
    XT[:, :, :N] = np.transpose(Xs, (0, 2, 3, 1)).reshape(S, C, N)
    x7 = XT.reshape(QUADS, 2, 2, C, NB, 128)  # [sq, r, h, cin, nb, node]
    XB = np.ascontiguousarray(np.transpose(x7, (4, 2, 3, 0, 1, 5)))
    XB = XB.reshape(NB, 128, F).astype(ml_dtypes.float8_e4m3)

    # W1 quad blockdiag packed for DoubleRow:
    # W4[(h,cin), r, r'*128 + h'*64 + cout] = W1s[cin,cout] iff r==r',h==h'
    W1s = (W1 * W1SCALE).astype(np.float32)
    W4 = np.zeros((2, 64, 2, 2, 2, 64), np.float32)  # h,cin,r,r',h',cout
    for h in range(2):
        for r in range(2):
            W4[h, :, r, r, h, :] = W1s
    W1d = np.ascontiguousarray(
        np.transpose(W4, (0, 1, 2, 3, 4, 5))).reshape(128, 512)
    W1d = W1d.astype(ml_dtypes.float8_e4m3)

    def blockdiag(W, dtype):
        D = np.zeros((128, 128), np.float32)
        D[:64, :64] = W
        D[64:, 64:] = W
        return D.astype(dtype)

    W2d = blockdiag(W2, ml_dtypes.bfloat16)
    B1 = np.tile(b1, (128, 512 // C)).astype(np.float32)
    B2 = np.concatenate([b2, b2])[:, None].astype(np.float32)

    in_maps = []
    for c in range(N_CORES):
        # MT slab: [NCH][128 src][j2][k][dst-chunk] fp8 ints
        MTc = Mfull[c * BPC * 128:(c + 1) * BPC * 128, :].T  # [NP src, 1280]
        MTc = MTc.reshape(NB2, 2, 128, NCH, CHW)
        MTc = np.ascontiguousarray(np.transpose(MTc, (3, 2, 0, 1, 4)))
        MTc = MTc.reshape(NCH, 128, NB2 * 2 * CHW)
        MTc = MTc.astype(ml_dtypes.float8_e4m3)

        DIc = dinv_pad[c * BPC * 128:(c + 1) * BPC * 128]
        DI = np.ascontiguousarray(DIc.reshape(BPC, 128).T.astype(np.float32))
        DI8 = np.ascontiguousarray(DI / W1SCALE)
        DB = np.ascontiguousarray(
            np.tile(DIc[None, :], (128, 1)).astype(np.float32))
        in_maps.append({"XB8": XB, "MT": MTc, "W1d": W1d, "W2d": W2d,
                        "B1": B1, "B2": B2, "DI": DI, "DI8": DI8,
                        "DB": DB})
    return in_maps


_NC_CACHE = {}


def kernel(X, edge_index, W1, b1, W2, b2):
    if "nc" not in _NC_CACHE:
        _NC_CACHE["nc"] = build_program(with_collective=True)
    nc = _NC_CACHE["nc"]
    in_maps = prepare_inputs(X, edge_index, W1, b1, W2, b2)

    res = None
    for attempt in range(5):
        try:
            res = run_bass_kernel_spmd(nc, in_maps, list(range(N_CORES)))
            break
        except Exception:
            if attempt == 4:
                raise
            time.sleep(60.0 * (attempt + 1))
    assert res is not None

    # reassemble: per core [12, 128, 1280] -> [24, 64, 1280]
    full = np.zeros((S, C, N), np.float32)
    for c in range(N_CORES):
        o = res.results[c]["OUT"].reshape(S, C, BPC * 128)
        lo = c * BPC * 128
        hi = min(N, (c + 1) * BPC * 128)
        if lo < N:
            full[:, :, lo:hi] = o[:, :, :hi - lo]
    out = full.reshape(B, T, C, N).transpose(0, 3, 1, 2)
    return np.ascontiguousarray(out)


# revision 25
# speedup vs baseline: 1.0809x; 1.0809x over previous
"""GCN block (2-layer) Trainium2 Bass kernel, v3.

Math (per B*T slice, shared graph):
  t2 = relu(A @ (X @ W1) + b1);  out = sigmoid(A @ t2 @ W2 + b2)
  A = D^-1/2 (Adj + I) D^-1/2  (PyG gcn_norm, counts edge multiplicity)

Device mapping (all-fp8 PE pipeline, M = Adj + I exact small ints in fp8):
  W1 : DoubleRow K=256 over slice-QUADS - stationary = X quad blocks
       [128=(h,cin), r, node], moving = blockdiag4(W1) [128, r, 256].
       One matmul produces a full 256-wide (2-pl) output chunk with the
       contraction finished, so W1 PE work halves vs the K=128 form.
       PSUM rotates banks 0-4; drains split DVE/ACT/Pool 3 ways (the
       drain engines, not PE, pace this era together with the X DMA).
  ck0: checkpoint-0's L1 runs DURING W1 as a 3-chain subpass (banks
       5-7, block h2=0) consuming xwp pairs a few steps behind
       production; block h2=1 runs as a second 3-chain subpass right
       after.  This fills the X-DMA-paced W1 era with L1 work.
  L1 : ckpts 1-4 - 6 chains on banks 0-5 in two trios (skew 0/1) so
       drains land 3-wide on DVE/Pool/ACT and xwp slots release at
       round i+1 for the t2c read handoff.  MT streams in HALF-chunk
       tiles (ring of 3) with natural pair order (half A pairs 0-19
       first), which shrinks the MT ring 40->30 KB/partition.
  t2 : one AllGather per checkpoint; 8 early t2c reads (group 0) land
       in the retired xb ring + a dedicated escape pool during ckpts
       1-3; groups 1-3 read into xwp slots as ckpt4 releases them;
       group 4 (half-A first) right after the last exchange.  The
       first L2 pass consumes pairs in ORD (arrival) order, so reads
       stay ahead of the chains and the read stall ~vanishes.
  L2 : FLIPPED orientation - stationary = t2 pair tiles (node-major),
       moving = MT dst-chunk slabs, fp8 DoubleRow; 12 chains run as 6
       co-banked PAIRS (pl 2p/2p+1 share bank p, same skew p, same end
       round, so the full bank drains at once - no half-bank-drain
       hazard).  W2 + sigmoid run on banks 6/7 inside the pass tail.
  W2 : stationary blockdiag(W2) bf16 over the drained bf16 s2 chunks,
       sigmoid+bias on ACT, fp32 tiles DMA'd to the output.

Sharding: each of 8 cores owns 10 of the 80 dst-node blocks (N padded
10000->10240) for ALL 24 B*T slices.  SBUF keeps the full xw / t2
operand set resident (40 pair tiles, 120 KB/partition); the same ring
is reused between layers (t2c tiles overwrite xwp slots).
"""
import time

import numpy as np
import ml_dtypes

import concourse.bacc as bacc
import concourse.mybir as mybir
import concourse.tile as tile
from concourse.bass_utils import run_bass_kernel_spmd

N_CORES = 8
N = 10000
NP = 10240            # padded nodes
NB = NP // 128        # 80 node blocks
NB2 = NB // 2         # 40 src-block pairs (DoubleRow K=256)
HNB2 = NB2 // 2       # 20 pairs per MT half tile
BPC = NB // N_CORES   # 10 dst blocks per core
NCK = BPC // 2        # 5 t2 checkpoints (dst-block pairs) per core
CHW = 256             # L2 dst-chunk width
NCH = BPC * 128 // CHW  # 5 dst chunks per core
B, T, C = 2, 12, 64
S = B * T             # 24 slices
F = S * C             # 1536 free columns
PAIRS = S // 2        # 12 slice pairs (pl)
QUADS = S // 4        # 6 slice quads (W1 DoubleRow groups)
CHAINS = ((0, 512), (512, 512), (1024, 512))
W1SCALE = 8.0         # W1 pre-scale so fp8 weights stay mostly normal
CK0LAG = 4            # ckpt0 subpass-A lag behind W1 production

f32 = mybir.dt.float32
bf16 = mybir.dt.bfloat16
fp8 = mybir.dt.float8e4
DR = mybir.MatmulPerfMode.DoubleRow
AF = mybir.ActivationFunctionType

# W1 drain engine rotation: ACT ~498ns, DVE ~595ns, Pool ~711ns per
# [128,512] chunk -> weights 7:6:5 balance the three engines.
ENG18 = "ADPADPADPADPADADAP"


def build_program(with_collective=True, nc_hook=None):
    nc = bacc.Bacc("TRN2", target_bir_lowering=False, debug=False,
                   num_devices=N_CORES)
    if nc_hook is not None:
        nc_hook(nc)

    # X quad blocks: [b][128=(h,cin)][sq*256 + r*128 + node] fp8,
    # dinv-src folded; slice s = 4*sq + 2*r + h
    xb_ext = nc.dram_tensor("XB8", [NB, 128, F], fp8, kind="ExternalInput")
    # MT column slabs: [chunk][128 src][j2*512 + k*256 + dst] fp8 ints
    mt_ext = nc.dram_tensor("MT", [NCH, 128, NB2 * 2 * CHW], fp8,
                            kind="ExternalInput")
    # blockdiag4(W1*8) packed for DoubleRow: [128=(h,cin), r*256 + col]
    w1_ext = nc.dram_tensor("W1d", [128, 512], fp8, kind="ExternalInput")
    w2_ext = nc.dram_tensor("W2d", [128, 128], bf16, kind="ExternalInput")
    b1_ext = nc.dram_tensor("B1", [128, 512], f32, kind="ExternalInput")
    b2_ext = nc.dram_tensor("B2", [128, 1], f32, kind="ExternalInput")
    di_ext = nc.dram_tensor("DI", [128, BPC], f32, kind="ExternalInput")
    d8_ext = nc.dram_tensor("DI8", [128, BPC], f32, kind="ExternalInput")
    db_ext = nc.dram_tensor("DB", [128, BPC * 128], f32,
                            kind="ExternalInput")
    out_ext = nc.dram_tensor("OUT", [PAIRS, 128, BPC * 128], f32,
                             kind="ExternalOutput")

    with tile.TileContext(nc) as tc:
        with (
            tc.tile_pool(name="consts", bufs=1) as consts,
            tc.tile_pool(name="xb", bufs=4) as pool_xb,
            tc.tile_pool(name="xwp", bufs=NB2) as pool_xwp,
            tc.tile_pool(name="esc", bufs=5) as pool_esc,
            tc.tile_pool(name="mt", bufs=3) as pool_mt,
            tc.tile_pool(name="u", bufs=2) as pool_u,
            tc.tile_pool(name="t2s", bufs=2) as pool_t2s,
            tc.tile_pool(name="s2", bufs=8) as pool_s2,
            tc.tile_pool(name="outst", bufs=3) as pool_out,
            tc.tile_pool(name="ps", bufs=8, space="PSUM") as pool_ps,
            tc.tile_pool(name="dram", bufs=1, space="DRAM") as dram,
        ):
            # startup-critical const only; the small drain consts ride
            # the ACT queue so the SP HWDGE goes straight to X blocks
            w1t = consts.tile([128, 2, 256], fp8, tag="w1")
            nc.sync.dma_start(w1t[:].rearrange("p r c -> p (r c)"),
                              w1_ext[:])
            b1t = consts.tile([128, 512], f32, tag="b1")
            nc.scalar.dma_start(b1t[:], b1_ext[:])
            dit = consts.tile([128, BPC], f32, tag="di")
            nc.scalar.dma_start(dit[:], di_ext[:])
            di8 = consts.tile([128, BPC], f32, tag="di8")
            nc.scalar.dma_start(di8[:], d8_ext[:])

            # DRAM intermediates: per-checkpoint t2 slabs
            t2loc = [dram.tile([2, 128, F], fp8, tag="t2loc",
                               name=f"t2loc{k}") for k in range(NCK)]
            if with_collective:
                t2full = [dram.tile([N_CORES, 2, 128, F], fp8, tag="t2full",
                                    name=f"t2full{k}", addr_space="Shared")
                          for k in range(NCK)]
            else:
                t2full = [dram.tile([N_CORES, 2, 128, F], fp8, tag="t2full",
                                    name=f"t2full{k}") for k in range(NCK)]

            # t2c handoff order: pairs grouped by checkpoint (j2 % NCK)
            ORD = [k + NCK * i for k in range(NCK) for i in range(NB2 // NCK)]

            def ps_tile(slot, name):
                return pool_ps.tile([128, 512], f32, tag=f"s{slot}",
                                    bufs=1, name=name)

            def mt_load(ch, half, eng=None):
                t = pool_mt.tile([128, HNB2, 2, CHW], fp8, tag="mt",
                                 name=f"mt{ch}{'AB'[half]}")
                off = half * HNB2 * 2 * CHW
                (eng or nc.sync).dma_start(
                    t[:].rearrange("p a b d -> p (a b d)"),
                    mt_ext[ch][:, off:off + HNB2 * 2 * CHW])
                return t

            def mt_pair(mth, i):
                # [128, 2, 256] slab for pair i from half tiles
                return mth[i // HNB2][:, i % HNB2, :, :]

            # xwp tiles ALLOCATED in ORD order so slot releases (which
            # fire in allocation order) line up with ckpt4's ORD-order
            # consumption and the t2c reads that reuse the slots.
            xwp = [None] * NB2
            for i in range(NB2):
                xwp[ORD[i]] = pool_xwp.tile([128, 2, F], fp8, tag="xwp",
                                            name=f"xwp{ORD[i]}")

            exchanged = []

            def exchange(ch, t2st):
                nc.gpsimd.dma_start(
                    t2loc[ch][:].rearrange("a p f -> p a f"), t2st[:])
                if with_collective:
                    nc.gpsimd.collective_compute(
                        "AllGather", mybir.AluOpType.bypass,
                        replica_groups=[list(range(N_CORES))],
                        ins=[t2loc[ch][:]], outs=[t2full[ch][:]])
                else:
                    # one 8-way broadcast DMA (0-stride src) models the
                    # same per-core traffic as the 8 slab writes but
                    # costs a single descriptor-gen pass
                    nc.gpsimd.dma_start(
                        t2full[ch][:],
                        t2loc[ch][:].unsqueeze(0)
                        .broadcast_to([N_CORES, 2, 128, F]))
                exchanged.append(ch)

            t2c = [None] * NB2

            def t2c_read(j2, t):
                gc = 2 * j2
                core, ck = gc // BPC, (gc % BPC) // 2
                assert ck in exchanged, (j2, ck)
                t2c[j2] = t
                nc.sync.dma_start(
                    t[:], t2full[ck][core].rearrange("a p f -> p a f"))

            # ---- W1 (DoubleRow), X-DMA paced -----------------------
            # Per block, 6 quad matmuls [128,256] with the contraction
            # done in one DR pass; PSUM rotates all 8 banks as
            # [128,512] (2-quad) chunks; drains split DVE/ACT/Pool.
            # No other PE work is interleaved: the era is X-DMA bound
            # and extra work here would run at the mid p-state (2x).
            nchunk = 0
            mth = {0: [None, None]}
            for j2 in range(NB2):
                if j2 == 28:
                    mth[0][0] = mt_load(0, 0)
                if j2 == 32:
                    mth[0][1] = mt_load(0, 1)
                xb = pool_xb.tile([128, 2, F], fp8, tag="xb",
                                  name=f"xb{j2}")
                nc.sync.dma_start(
                    xb[:], xb_ext[2 * j2:2 * j2 + 2]
                    .rearrange("a p d -> p a d"))
                xw = xwp[j2]
                for k in range(2):
                    for sq2 in range(3):
                        ps = ps_tile(nchunk % 8, f"w1p{nchunk}")
                        for q in range(2):
                            sq = 2 * sq2 + q
                            nc.tensor.matmul(
                                ps[:, q * 256:(q + 1) * 256],
                                xb[:, k, sq * 256:(sq + 1) * 256]
                                .rearrange("p (r n) -> p r n", r=2),
                                w1t[:], start=True, stop=True,
                                perf_mode=DR)
                        dst = xw[:, k, sq2 * 512:(sq2 + 1) * 512]
                        e = ENG18[nchunk % 18]
                        if e == "A":
                            nc.scalar.activation(dst, ps[:], AF.Copy)
                        elif e == "D":
                            nc.vector.tensor_scalar_mul(dst, ps[:], 1.0)
                        else:
                            nc.gpsimd.tensor_scalar_mul(dst, ps[:], 1.0)
                        nchunk += 1

            mth[1] = [mt_load(1, 0), mt_load(1, 1)]

            # ---- L1 ckpts 0-4: 6 chains as two trios (skew 0/1) -----
            for ch in range(NCK):
                t2st = pool_t2s.tile([128, 2, F], fp8, tag="t2s",
                                     name=f"t2s{ch}")
                order = (list(range(NB2)) if ch < NCK - 1
                         else [ORD[i] for i in range(NB2)])
                ps_list = [ps_tile(idx, f"pa{ch}_{idx}")
                           for idx in range(6)]
                for r in range(NB2 + 2):
                    for idx in range(6):
                        h2, skew = idx // 3, idx // 3
                        i = r - skew
                        if not 0 <= i < NB2:
                            continue
                        j2 = order[i]
                        c0, w = CHAINS[idx % 3]
                        nc.tensor.matmul(
                            ps_list[idx][:],
                            mt_pair(mth[ch], j2)[:, :,
                                                 h2 * 128:(h2 + 1) * 128],
                            xwp[j2][:, :, c0:c0 + w],
                            start=(i == 0), stop=(i == NB2 - 1),
                            perf_mode=DR)
                    if ch == NCK - 1 and 9 <= r - 2 < 32:
                        # slot ORD[r-2] released by trio B last round;
                        # read groups 1-3 into the freed xwp slots.
                        i2 = r - 2
                        j2r = ORD[i2]
                        t = pool_xwp.tile([128, 2, F], fp8, tag="xwp",
                                          name=f"t2c{j2r}")
                        t2c_read(j2r, t)
                    if 0 < ch < NCK - 1 and r == 21:
                        # prefetch next ckpt's MT halves: this ckpt's
                        # half-A readers (slot predecessors) are all
                        # emitted by round 21, so the WAR edges bind.
                        mth[ch + 1] = [mt_load(ch + 1, 0),
                                       mt_load(ch + 1, 1)]
                    for idx in range(6):
                        h2, skew = idx // 3, idx // 3
                        if r - skew != NB2 - 1:
                            continue
                        c0, w = CHAINS[idx % 3]
                        bi = 2 * ch + h2
                        u = pool_u.tile([128, 512], f32, tag="u")
                        eng = nc.vector if idx % 3 != 1 else nc.gpsimd
                        eng.scalar_tensor_tensor(
                            u[:], ps_list[idx][:], di8[:, bi:bi + 1],
                            b1t[:], mybir.AluOpType.mult,
                            mybir.AluOpType.add)
                        nc.scalar.activation(t2st[:, h2, c0:c0 + w], u[:],
                                             AF.Relu,
                                             scale=dit[:, bi:bi + 1])
                exchange(ch, t2st)
                if ch <= 1:
                    # 10 early t2c reads: group 0 (8) after exchange 0
                    # into the retired xb ring + escape pool; the first
                    # two group-1 pairs after exchange 1.  They fire as
                    # soon as their exchange lands, over ckpts 1-3.
                    for m in (range(8) if ch == 0 else range(8, 9)):
                        j2e = ORD[m]
                        if m < 4:
                            t = pool_xb.tile([128, 2, F], fp8, tag="xb",
                                             name=f"t2ce{j2e}")
                        else:
                            t = pool_esc.tile([128, 2, F], fp8, tag="esc",
                                              name=f"t2ce{j2e}")
                        t2c_read(j2e, t)
                if ch == 0:
                    # L2-era consts ride the ckpt1-window DMA slack
                    w2t = consts.tile([128, 128], bf16, tag="w2")
                    nc.sync.dma_start(w2t[:], w2_ext[:])
                    b2t = consts.tile([128, 1], f32, tag="b2")
                    nc.sync.dma_start(b2t[:], b2_ext[:])
                    dbt = consts.tile([128, BPC * 128], f32, tag="db")
                    nc.sync.dma_start(dbt[:], db_ext[:])

            # group-4 t2c reads (half-A pairs first = ORD order), into
            # the xwp slots ckpt4 released (allocation order continues)
            for i2 in range(32, NB2):
                j2r = ORD[i2]
                t = pool_xwp.tile([128, 2, F], fp8, tag="xwp",
                                  name=f"t2c{j2r}")
                t2c_read(j2r, t)

            # ---- L2 (flipped) + W2 + sigmoid, per dst chunk ---------
            # 12 chains = 6 co-banked pairs (pl 2p/2p+1 on bank p, skew
            # p, same end round -> full-bank drain).  W2 on banks 6/7.
            # Chunk NCH-1 first (MT resident, pairs in ORD/arrival
            # order); then descending with natural halfwise order and
            # half-granular MT reloads.
            for ch in range(NCH - 1, -1, -1):
                if ch == NCH - 1:
                    mtt = mth[NCK - 1]     # still resident from L1
                    order = [ORD[i] for i in range(NB2)]
                else:
                    order = list(range(NB2))
                    mtt = mth[ch]
                wt = [ps_tile(p, f"l2w{ch}_{p}") for p in range(6)]
                s2l = [None] * PAIRS
                for r in range(NB2 + 6):
                    for p in range(6):
                        i = r - p
                        if not 0 <= i < NB2:
                            continue
                        j2 = order[i]
                        for h in range(2):
                            pl = 2 * p + h
                            nc.tensor.matmul(
                                wt[p][:, h * CHW:(h + 1) * CHW],
                                t2c[j2][:, :, pl * 128:(pl + 1) * 128],
                                mt_pair(mtt, j2)[:],
                                start=(i == 0), stop=(i == NB2 - 1),
                                perf_mode=DR)
                    p = r - (NB2 - 1)
                    if 0 <= p < 6:
                        # full-bank ready: drain both halves (s2 = bf16
                        # dinv_dst fold); W2 is deferred past the pass
                        for h in range(2):
                            pl = 2 * p + h
                            s2 = pool_s2.tile([128, CHW], bf16, tag="s2")
                            eng = nc.vector if h == 0 else nc.gpsimd
                            eng.tensor_tensor(
                                s2[:], wt[p][:, h * CHW:(h + 1) * CHW],
                                dbt[:, ch * CHW:(ch + 1) * CHW],
                                mybir.AluOpType.mult)
                            s2l[pl] = s2
                # deferred W2 + sigmoid block: the W2 psums reuse the
                # chain banks (drained above), so no W2 matmul ever
                # stalls the in-order PE stream mid-pass, and the next
                # chunk's chains WAR only on the sigmoids.
                for p in range(6):
                    w2ps = ps_tile(p, f"w2p{ch}_{p}")
                    for h in range(2):
                        nc.tensor.matmul(w2ps[:, h * CHW:(h + 1) * CHW],
                                         w2t[:], s2l[2 * p + h][:],
                                         start=True, stop=True)
                    # one full-bank sigmoid per pl-pair (b2 bias is
                    # per-partition, identical for both halves)
                    outst = pool_out.tile([128, 2, CHW], f32, tag="outst")
                    nc.scalar.activation(
                        outst[:].rearrange("p a d -> p (a d)"), w2ps[:],
                        AF.Sigmoid, bias=b2t[:])
                    nc.sync.dma_start(
                        out_ext[2 * p:2 * p + 2, :,
                                ch * CHW:(ch + 1) * CHW]
                        .rearrange("a p d -> p a d"), outst[:])
                if ch > 0:
                    # reload next chunk's halves; emitted after the full
                    # round loop so every reader of the recycled ring
                    # slots exists before the WAR edge is formed.  The
                    # ACT queue pins them behind this chunk's sigmoids,
                    # clear of the t2c read stream.
                    mth[ch - 1] = [mt_load(ch - 1, 0), mt_load(ch - 1, 1)]

    nc.compile()
    return nc


def prepare_inputs(X, edge_index, W1, b1, W2, b2):
    """Host-side graph/layout prep. Returns per-core in_maps."""
    X = np.asarray(X, dtype=np.float32)
    edge_index = np.asarray(edge_index)
    W1 = np.asarray(W1, dtype=np.float32)
    b1 = np.asarray(b1, dtype=np.float32)
    W2 = np.asarray(W2, dtype=np.float32)
    b2 = np.asarray(b2, dtype=np.float32)

    src = edge_index[0].astype(np.int64)
    dst = edge_index[1].astype(np.int64)

    deg = np.bincount(dst, minlength=N).astype(np.float32) + 1.0
    dinv = 1.0 / np.sqrt(deg)
    dinv_pad = np.zeros(NP, np.float32)
    dinv_pad[:N] = dinv

    # M = Adj + I with multiplicity, uint8 counts
    Mfull = np.zeros((NP, NP), np.uint8)
    np.add.at(Mfull, (dst, src), 1)
    Mfull[np.arange(N), np.arange(N)] += 1
    assert Mfull.max() <= 15, "fp8e4 exact-int range exceeded"

    # XB: [NB, 128=(h,cin), sq*256 + r*128 + node] fp8, dinv-src folded;
    # slice s = 4*sq + 2*r + h
    Xs = X * dinv[None, :, None, None]                  # [B, N, T, C]
    XT = np.zeros((S, C, NP), np.float32)
    XT[:, :, :N] = np.transpose(Xs, (0, 2, 3, 1)).reshape(S, C, N)
    x7 = XT.reshape(QUADS, 2, 2, C, NB, 128)  # [sq, r, h, cin, nb, node]
    XB = np.ascontiguousarray(np.transpose(x7, (4, 2, 3, 0, 1, 5)))
    XB = XB.reshape(NB, 128, F).astype(ml_dtypes.float8_e4m3)

    # W1 quad blockdiag packed for DoubleRow:
    # W4[(h,cin), r, r'*128 + h'*64 + cout] = W1s[cin,cout] iff r==r',h==h'
    W1s = (W1 * W1SCALE).astype(np.float32)
    W4 = np.zeros((2, 64, 2, 2, 2, 64), np.float32)  # h,cin,r,r',h',cout
    for h in range(2):
        for r in range(2):
            W4[h, :, r, r, h, :] = W1s
    W1d = np.ascontiguousarray(
        np.transpose(W4, (0, 1, 2, 3, 4, 5))).reshape(128, 512)
    W1d = W1d.astype(ml_dtypes.float8_e4m3)

    def blockdiag(W, dtype):
        D = np.zeros((128, 128), np.float32)
        D[:64, :64] = W
        D[64:, 64:] = W
        return D.astype(dtype)

    W2d = blockdiag(W2, ml_dtypes.bfloat16)
    B1 = np.tile(b1, (128, 512 // C)).astype(np.float32)
    B2 = np.concatenate([b2, b2])[:, None].astype(np.float32)

    in_maps = []
    for c in range(N_CORES):
        # MT slab: [NCH][128 src][j2][k][dst-chunk] fp8 ints
        MTc = Mfull[c * BPC * 128:(c + 1) * BPC * 128, :].T  # [NP src, 1280]
        MTc = MTc.reshape(NB2, 2, 128, NCH, CHW)
        MTc = np.ascontiguousarray(np.transpose(MTc, (3, 2, 0, 1, 4)))
        MTc = MTc.reshape(NCH, 128, NB2 * 2 * CHW)
        MTc = MTc.astype(ml_dtypes.float8_e4m3)

        DIc = dinv_pad[c * BPC * 128:(c + 1) * BPC * 128]
        DI = np.ascontiguousarray(DIc.reshape(BPC, 128).T.astype(np.float32))
        DI8 = np.ascontiguousarray(DI / W1SCALE)
        DB = np.ascontiguousarray(
            np.tile(DIc[None, :], (128, 1)).astype(np.float32))
        in_maps.append({"XB8": XB, "MT": MTc, "W1d": W1d, "W2d": W2d,
                        "B1": B1, "B2": B2, "DI": DI, "DI8": DI8,
                        "DB": DB})
    return in_maps


_NC_CACHE = {}


def kernel(X, edge_index, W1, b1, W2, b2):
    if "nc" not in _NC_CACHE:
        _NC_CACHE["nc"] = build_program(with_collective=True)
    nc = _NC_CACHE["nc"]
    in_maps = prepare_inputs(X, edge_index, W1, b1, W2, b2)

    res = None
    for attempt in range(5):
        try:
            res = run_bass_kernel_spmd(nc, in_maps, list(range(N_CORES)))
            break
        except Exception:
            if attempt == 4:
                raise
            time.sleep(60.0 * (attempt + 1))
    assert res is not None

    # reassemble: per core [12, 128, 1280] -> [24, 64, 1280]
    full = np.zeros((S, C, N), np.float32)
    for c in range(N_CORES):
        o = res.results[c]["OUT"].reshape(S, C, BPC * 128)
        lo = c * BPC * 128
        hi = min(N, (c + 1) * BPC * 128)
        if lo < N:
            full[:, :, lo:hi] = o[:, :, :hi - lo]
    out = full.reshape(B, T, C, N).transpose(0, 3, 1, 2)
    return np.ascontiguousarray(out)
